# revision 73
# baseline (speedup 1.0000x reference)
"""Trainium2 Bass kernel for the ULA beamformer DOA problem.

Contract: kernel(**inputs) takes FULL unsharded inputs (B=128 batch), shards
batch across 8 NeuronCores, runs a Bass/Tile kernel per core, and returns the
full (B, M) float32 angle labels.

Device algorithm per core (16 batch items):
  1. Load XP_b = [Xr_b; Xi_b] (128 x 1024), PE-transpose 128-col chunks.
  2. Split transposed chunks into fp16 hi/lo planes (PSUM evacuation copy +
     subtract).  Gram G_b = Z^T Z (128 x 128) per batch accumulated in PSUM:
     3 fp16 matmuls per chunk (hi*hi, hi*lo, lo*hi) at 128-wide, which the PE
     runs at 1 cycle/row at any width (fp32r would need >=256-wide, forcing
     the old scheme to waste half its FLOPs on cross-batch products).
  3. Toeplitz reduction: the ULA spectrum only needs the diagonal sums of the
     Hermitian covariance; cos/sin tables are the steering rows themselves.
     Diagonals are extracted with a stride-129 DRAM access pattern (both
     batches of a pair in one 3-level-AP DMA), masked and signed on DVE,
     column-summed with a ones-matmul -> W (128 x 16).
  4. spectrum = W^T @ SS_scaled  (one matmul, 16 x 3600).
  5. Peak detect (>= left, > right) + top-8 via DVE max/max_index.

Host: top-M selection from device top-8, with fp64 refinement of numerically
risky candidates (flat-top / near-tie cases) using the reference's direct
quadratic form.
"""

import numpy as np

B, N, T, A = 128, 64, 1024, 3600
NCORES = 8
BL = B // NCORES  # 16 batch items per core
BIG = np.float32(1e30)
RISK_RANK = 3e-5
RISK_FLAT = 2e-5

_cache = {}


def _host_constants():
    ident = np.eye(128, dtype=np.float32)
    masksgn = np.zeros((128, 128), np.float32)
    for p in range(128):
        n = p % 64
        for dp in range(128):
            d = dp % 64
            if n + d > 63:
                continue
            masksgn[p, dp] = 1.0 if dp < 64 else (-1.0 if p < 64 else 1.0)
    # fold the spectrum's per-diagonal scaling (1/T, 2/T, -2/T by column d)
    # into the mask: wcol[d] = sum_p upt[p,d]*msk[p,d], so scaling msk's
    # column d scales W row d -- no on-device scaling op needed at all
    scale = np.zeros(128, np.float32)
    scale[0] = 1.0 / T
    scale[1:64] = 2.0 / T
    scale[64:] = -2.0 / T
    masksgn *= scale[None, :]
    # aux = [masksgn(b1) | masksgn(b2) | ones]  (128 x 258)
    aux = np.zeros((128, 258), np.float32)
    aux[:, 0:128] = masksgn
    aux[:, 128:256] = masksgn
    aux[:, 256] = 1.0
    return ident, aux


def build_program(loop_n=None):
    """Build and compile the per-core Bass program. Returns the Bacc instance.
    loop_n wraps the whole body in an on-device loop (benchmarking only)."""
    key = ("nc", loop_n)
    if key in _cache:
        return _cache[key]
    from contextlib import ExitStack

    import concourse.bacc as bacc
    import concourse.mybir as mybir
    from concourse import tile
    from concourse.ap import AP

    f32 = mybir.dt.float32
    f32r = mybir.dt.float32r
    f16 = mybir.dt.float16
    u32 = mybir.dt.uint32

    nc = bacc.Bacc("TRN2", target_bir_lowering=False, debug=False)

    xr_d = nc.dram_tensor("xr", (BL, N, T), f32, kind="ExternalInput")
    xi_d = nc.dram_tensor("xi", (BL, N, T), f32, kind="ExternalInput")
    sr_d = nc.dram_tensor("sr", (N, A), f32, kind="ExternalInput")
    si_d = nc.dram_tensor("si", (N, A), f32, kind="ExternalInput")
    ident_d = nc.dram_tensor("ident", (128, 128), f32, kind="ExternalInput")
    aux_d = nc.dram_tensor("aux", (128, 258), f32, kind="ExternalInput")

    out_spec = nc.dram_tensor("out_spec", (BL, A), f32, kind="ExternalOutput")
    # top-8 per (eighth-chunk, batch) row; host merges the 8 chunks per batch
    out_idx = nc.dram_tensor("out_idx", (8 * BL, 8), u32, kind="ExternalOutput")
    out_val = nc.dram_tensor("out_val", (8 * BL, 8), f32, kind="ExternalOutput")

    # per-pair G scratch: (half, 130 rows, 128) -- rows 128:130 are pad so the
    # stride-129 diagonal reload stays in bounds
    g_dram = [nc.dram_tensor(f"gscr{p}", (2, 130, 128), f32)
              for p in range(BL // 2)]
    GH = 130 * 128  # element stride between the two halves of a pair scratch

    with tile.TileContext(nc) as tc, ExitStack() as ctx:
        const = ctx.enter_context(tc.tile_pool(name="const", bufs=1))
        xp_pool = ctx.enter_context(tc.tile_pool(name="xp", bufs=6))
        z_pool = ctx.enter_context(tc.tile_pool(name="z", bufs=2))
        g_pool = ctx.enter_context(tc.tile_pool(name="g", bufs=4))
        up_pool = ctx.enter_context(tc.tile_pool(name="up", bufs=4))
        w_pool = ctx.enter_context(tc.tile_pool(name="w", bufs=1))
        spec_pool = ctx.enter_context(tc.tile_pool(name="spec", bufs=1))
        pz = ctx.enter_context(tc.tile_pool(name="pz", bufs=3, space="PSUM"))
        pg = ctx.enter_context(tc.tile_pool(name="pg", bufs=3, space="PSUM"))
        ps = ctx.enter_context(tc.tile_pool(name="ps", bufs=2, space="PSUM"))

        ident_t = const.tile([128, 128], f32)
        aux_t = const.tile([128, 258], f32)
        msk2_t = aux_t[:, 0:256]
        ones_t = aux_t[:, 256:257]

        # ---- input loads: one DMA per (pair, re/im plane); pair 0 is split
        # finer so the first transposes can start on a quarter of the data
        def load_into(xpp, pair):
            b1 = 2 * pair
            xr2 = xr_d.ap()[b1:b1 + 2].rearrange("b n t -> n b t")
            xi2 = xi_d.ap()[b1:b1 + 2].rearrange("b n t -> n b t")
            xpp_r = xpp[0:64, :].rearrange("p (b t) -> p b t", b=2)
            xpp_i = xpp[64:128, :].rearrange("p (b t) -> p b t", b=2)
            nc.sync.dma_start(xpp_r, xr2)
            nc.sync.dma_start(xpp_i, xi2)

        def load_pair_inputs(pair):
            b1 = 2 * pair
            xpp = xp_pool.tile([128, 2 * T], f32, tag="xp", name=f"xpp{pair}")
            if pair == 0 and loop_n is None:
                # b1 halves first (ident rides the Pool queue in parallel),
                # then b2
                nc.sync.dma_start(xpp[0:64, 0:512], xr_d.ap()[b1, :, 0:512])
                nc.sync.dma_start(xpp[64:128, 0:512], xi_d.ap()[b1, :, 0:512])
                nc.sync.dma_start(xpp[0:64, 512:T], xr_d.ap()[b1, :, 512:T])
                nc.sync.dma_start(xpp[64:128, 512:T], xi_d.ap()[b1, :, 512:T])
                nc.sync.dma_start(
                    xpp[0:64, T:2 * T], xr_d.ap()[b1 + 1, :, :])
                nc.sync.dma_start(
                    xpp[64:128, T:2 * T], xi_d.ap()[b1 + 1, :, :])
            else:
                load_into(xpp, pair)
            return xpp

        nc.gpsimd.dma_start(ident_t[:], ident_d.ap())
        xpp_next = load_pair_inputs(0) if loop_n is None else None
        nc.gpsimd.dma_start(aux_t[:], aux_d.ap())

        # pad gscratch rows 128:130 once (values masked out later; must just
        # not be NaN) -- one small Pool DMA per pair, off the critical chain
        for p in range(BL // 2):
            nc.gpsimd.dma_start(
                AP(g_dram[p], 128 * 128, [[GH, 2], [128, 2], [1, 128]]),
                ident_t[0:4, :],
            )

        # SS_scaled: rows 0..63 = steer_real * (2-d0)/T, rows 64.. = steer_imag
        # * -2/T, then split into fp32r hi/lo planes for the 1-cycle/row
        # spectrum matmuls. Emitted mid-pipeline so the big engine ops don't
        # block the early pairs' PSUM evacuations.
        SS_t = const.tile([128, A], f32)
        SShi_t = const.tile([128, A], f32r)
        SSlo_t = const.tile([128, A], f32r)

        def emit_ss_load(part):
            # sixteenth-row chunks, two DMAs per pair iteration, so the
            # steering stream never backs up the serialized DMA engines
            # against the input loads
            lo = 16 * part
            nc.gpsimd.dma_start(SS_t[lo:lo + 16, :], sr_d.ap()[lo:lo + 16])
            nc.gpsimd.dma_start(
                SS_t[64 + lo:80 + lo, :], si_d.ap()[lo:lo + 16]
            )

        def emit_ss_prep(part):
            # quarter-column chunks spread over the early pair iterations so
            # no single op blocks the ACT/DVE queues while the pipeline runs
            lo, hi = 900 * part, 900 * (part + 1)
            nc.scalar.copy(SShi_t[:, lo:hi], SS_t[:, lo:hi])
            nc.vector.tensor_tensor(
                SSlo_t[:, lo:hi], SS_t[:, lo:hi], SShi_t[:, lo:hi],
                op=mybir.AluOpType.subtract,
            )

        # W columns for all 16 batches: the spectrum runs as ONE pass with a
        # [128, 16] stationary, halving its PE time vs per-half passes and
        # keeping every spectral op out of the pair pipeline's engine queues
        W_t = w_pool.tile([128, BL], f32, name="W")

        # peak-mask tile; non-peaks and the border columns become 0.0, which
        # ranks below every real peak (spectrum values here are all >> 0)
        mskf_t = spec_pool.tile([128, 452], f32, name="mskf")
        nc.gpsimd.memset(mskf_t[:], 0.0)

        def spectrum_begin():
            # spectrum rows for all 16 batches, reshaped on the fly to
            # (128, 452) with one-column halos: peak detection uses all 128
            # partitions. Row 16*j + i <-> batch i, eighth j; local column l
            # <-> global angle a = 450*j - 1 + l. Border pads are written
            # first (+BIG so a=0 / a=A-1 never count as peaks); the halo DMAs
            # overwrite the pad cells of interior eighths.
            spec_t = spec_pool.tile([BL, A], f32, name="spec")
            sp4 = spec_pool.tile([128, 452], f32, name="sp4")
            nc.gpsimd.memset(sp4[:, 0:1], float(BIG))
            nc.gpsimd.memset(sp4[:, 451:452], float(BIG))
            # W hi/lo planes for the fp32r spectrum matmuls
            whi = spec_pool.tile([128, BL], f32r, name="whi")
            wlo = spec_pool.tile([128, BL], f32r, name="wlo")
            nc.scalar.copy(whi[:], W_t[:])
            nc.vector.tensor_tensor(
                wlo[:], W_t[:], whi[:], op=mybir.AluOpType.subtract
            )
            return {"spec": spec_t, "sp4": sp4, "whi": whi, "wlo": wlo}

        def spectrum_chunks(st, chunks):
            # eighth-aligned 452-wide chunks (overlapping by 2 columns) so
            # each eighth's sp4 reshape fires right after its own chunk's
            # evacuation instead of waiting on a neighbor
            spec_t, sp4 = st["spec"], st["sp4"]

            def reshape(j):
                if j == 0:
                    nc.sync.dma_start(sp4[0:16, 1:452], spec_t[:, 0:451])
                elif j == 7:
                    nc.sync.dma_start(sp4[112:128, 0:451], spec_t[:, 3149:3600])
                else:
                    nc.sync.dma_start(
                        sp4[16 * j:16 * j + 16, 0:452],
                        spec_t[:, 450 * j - 1:450 * j + 451],
                    )

            for k in chunks:
                off = 450 * k
                cw = min(452, A - off)
                pst = ps.tile([BL, cw], f32, tag="ps", name=f"ps_{off}")
                nc.tensor.matmul(pst[:], st["whi"][:], SShi_t[:, off:off + cw],
                                 start=True, stop=False)
                nc.tensor.matmul(pst[:], st["whi"][:], SSlo_t[:, off:off + cw],
                                 start=False, stop=False)
                nc.tensor.matmul(pst[:], st["wlo"][:], SShi_t[:, off:off + cw],
                                 start=False, stop=True)
                # skip the 2 overlap columns chunk k-1 already wrote (same
                # values): a WAR on them would chain each evacuation behind
                # the previous chunk's reshape DMA completion
                sk = 0 if k == 0 else 2
                nc.scalar.copy(spec_t[:, off + sk:off + cw], pst[:, sk:cw])
                reshape(k)
            if chunks[-1] == 7:
                # Pool queue: off the critical tail, parallel to the HWDGE
                # path carrying the idx/val outputs
                nc.gpsimd.dma_start(out_spec.ap(), spec_t[:])

        def peaks_compare(st, rows, eng):
            # at-least-left, strictly-above-right: flat tops keep their
            # rightmost member so near-equal peaks are not annihilated.
            # rows 0:64 ride Pool concurrently while spectrum chunks 4..7
            # stream; rows 64:128 are the latency-critical tail and ride the
            # faster DVE.
            sp4 = st["sp4"]
            r0, r1 = rows
            m1u = spec_pool.tile([128, 450], f32, name="m1u")
            m2u = spec_pool.tile([128, 450], f32, name="m2u")
            eng.tensor_tensor(
                m1u[r0:r1, :], sp4[r0:r1, 1:451], sp4[r0:r1, 0:450],
                op=mybir.AluOpType.is_ge,
            )
            eng.tensor_tensor(
                m2u[r0:r1, :], sp4[r0:r1, 1:451], sp4[r0:r1, 2:452],
                op=mybir.AluOpType.is_gt,
            )
            eng.tensor_mul(m1u[r0:r1, :], m1u[r0:r1, :], m2u[r0:r1, :])
            eng.tensor_mul(
                mskf_t[r0:r1, 1:451], m1u[r0:r1, :], sp4[r0:r1, 1:451]
            )

        val8_t = spec_pool.tile([128, 8], f32, name="val8")
        idx8_t = spec_pool.tile([128, 8], u32, name="idx8")

        def peaks_max(rows):
            # by the time these run the sync queue has drained its reshapes,
            # so the HWDGE path gives the lowest-latency output DMAs
            r0, r1 = rows
            nc.vector.max(val8_t[r0:r1, :], mskf_t[r0:r1, :])
            nc.vector.max_index(idx8_t[r0:r1, :], val8_t[r0:r1, :], mskf_t[r0:r1, :])
            nc.sync.dma_start(out_idx.ap()[r0:r1, :], idx8_t[r0:r1, :])
            nc.sync.dma_start(out_val.ap()[r0:r1, :], val8_t[r0:r1, :])

        def alloc_z(pair):
            # z layout per pair: [b1 chunks 0..7 | b2 chunks 0..7], 1024 cols
            # each, split into fp16 hi/lo planes: G = Zh^T Zh + Zh^T Zl +
            # Zl^T Zh runs the PE at 1 cycle/row at 128-wide with ~2^-21
            # precision.
            zh = z_pool.tile([128, 2048], f16, tag="zh", name=f"zh{pair}")
            zl = z_pool.tile([128, 2048], f16, tag="zl", name=f"zl{pair}")
            return zh, zl

        def emit_transpose_group(pair, xpp, zq, h, q):
            # transposes pack 4 chunks of half h into one PSUM bank tile;
            # the fp16 hi plane is the (rounding) PSUM evacuation copy, the
            # lo plane is one extra subtract
            zh, zl = zq
            pzt = pz.tile([128, 512], f32, tag="pz", name=f"pz{pair}{q}{h}")
            for k in range(4):
                c = 4 * q + k
                nc.tensor.matmul(
                    pzt[:, k * 128:(k + 1) * 128],
                    xpp[:, h * T + c * 128:h * T + (c + 1) * 128],
                    ident_t[:],
                    is_transpose=True,
                    start=(k == 0),
                    stop=(k == 3),
                )
            lo = h * 1024 + q * 512
            nc.scalar.copy(zh[:, lo:lo + 512], pzt[:])
            nc.vector.tensor_tensor(
                zl[:, lo:lo + 512], pzt[:], zh[:, lo:lo + 512],
                op=mybir.AluOpType.subtract,
            )

        def emit_transposes(pair, xpp, zq):
            # h-major group order: batch h's evacuation groups complete
            # back-to-back, matching the batch-major gram consumption order
            for h in (0, 1):
                for q in (0, 1):
                    emit_transpose_group(pair, xpp, zq, h, q)

        gts = {}
        upts = {}

        def emit_gram_part(pair, zh, zl, h, q):
            # both G's of the pair live in one PSUM bank: [G(b1) | G(b2) |
            # wcol(b1) | wcol(b2)]; the first matmul's start=True marks the
            # whole 2KB bank pending-zero, so each region's first write
            # overwrites and the rest accumulate. Batch-major order so the
            # first batch's G (and its diag round-trip) completes while the
            # second batch's gram is still streaming.
            if h == 0 and q == 0:
                gts[pair] = pg.tile([128, 258], f32, tag="gt", name=f"gt{pair}")
            gt = gts[pair]
            n_mm = 24 * h + 12 * q
            for c in range(4 * q, 4 * q + 4):
                lo = h * 1024 + c * 128
                zhc = zh[:, lo:lo + 128]
                zlc = zl[:, lo:lo + 128]
                out = gt[:, h * 128:(h + 1) * 128]
                for lhsT, rhs in ((zhc, zhc), (zhc, zlc), (zlc, zhc)):
                    nc.tensor.matmul(
                        out, lhsT, rhs,
                        start=(n_mm == 0),
                        stop=(n_mm == 47),
                    )
                    n_mm += 1

        def emit_grams_half(pair, h, zh, zl):
            emit_gram_part(pair, zh, zl, h, 0)
            emit_gram_part(pair, zh, zl, h, 1)

        def emit_grams(pair, zh, zl):
            emit_grams_half(pair, 0, zh, zl)
            emit_grams_half(pair, 1, zh, zl)

        def emit_diag_write(pair):
            # G -> DRAM; no PE work, so it can chase the grams immediately.
            # Both batches of the pair ride single 3-level-AP DMAs.
            gt = gts[pair]
            gd = g_dram[pair]
            gsb = g_pool.tile([128, 256], f32, tag="g", name=f"gsb{pair}")
            nc.scalar.copy(gsb[:], gt[:, 0:256])
            nc.sync.dma_start(
                AP(gd, 0, [[128, 128], [GH, 2], [1, 128]]),
                gsb[:].rearrange("p (h c) -> p h c", h=2),
            )

        def emit_diag_read(pair, mask_eng=None):
            # stride-129 diagonal reload -> signed mask. Mid-pipeline masks
            # ride Pool (keeping the DVE queue clear for the PSUM-evacuation
            # subtracts); the tail pairs' masks stay on the faster DVE.
            gd = g_dram[pair]
            upt = up_pool.tile([128, 256], f32, tag="up", name=f"up{pair}")
            upts[pair] = upt
            nc.sync.dma_start(
                upt[:].rearrange("p (h c) -> p h c", h=2),
                AP(gd, 0, [[129, 128], [GH, 2], [1, 128]]),
            )
            nc.sync.dma_start(
                upt[64:128, :].rearrange("p (h c) -> p h c", h=2)[:, :, 64:128],
                AP(gd, 64 * 128, [[129, 64], [GH, 2], [1, 64]]),
            )
            (mask_eng or nc.vector).tensor_mul(upt[:], upt[:], msk2_t)

        def emit_diag_start(pair):
            emit_diag_write(pair)
            emit_diag_read(pair)

        def emit_diag_write_b(pair, h):
            # per-batch variant for the last pair: batch h's write launches
            # while batch h+1's gram is still on the PE
            gt = gts[pair]
            gd = g_dram[pair]
            gsb = g_pool.tile([128, 128], f32, tag=f"gb{h}", name=f"gsb{pair}_{h}")
            nc.scalar.copy(gsb[:], gt[:, h * 128:(h + 1) * 128])
            nc.sync.dma_start(AP(gd, h * GH, [[128, 128], [1, 128]]), gsb[:])

        def emit_diag_read_b(pair, h):
            # reads ride the ACT HWDGE queue (idle in the tail): while one
            # read holds the queue head waiting for its write's semaphore,
            # the sync queue keeps issuing the other batch's write
            gd = g_dram[pair]
            upt = up_pool.tile([128, 128], f32, tag=f"ub{h}", name=f"up{pair}_{h}")
            upts[(pair, h)] = upt
            nc.sync.dma_start(upt[:], AP(gd, h * GH, [[129, 128], [1, 128]]))
            nc.sync.dma_start(
                upt[64:128, 64:128], AP(gd, h * GH + 64 * 128, [[129, 64], [1, 64]])
            )
            nc.vector.tensor_mul(upt[:], upt[:], msk2_t[:, 0:128])

        def emit_diag_finish(pair, split=False):
            # column-sums (PE) one pipeline stage later, when the DRAM
            # round-trip has surely completed
            gt = gts.pop(pair)
            for h in (0, 1):
                b = 2 * pair + h
                upt = upts.pop((pair, h)) if split else upts.get(pair)
                ucol = upt[:, 0:128] if split else upt[:, h * 128:(h + 1) * 128]
                wcol = gt[:, 256 + h:257 + h]
                nc.tensor.matmul(wcol, ucol, ones_t)
                nc.scalar.copy(W_t[:, b:b + 1], wcol)
            upts.pop(pair, None)

        # software pipeline: transposes of pair p run on the PE while the
        # grams of pair p-1 stream and the diagonal extraction of pair p-2
        # finishes, hiding both the PSUM-evacuation (copy+subtract) latency
        # and the DRAM diagonal round-trip
        def emit_body(inline_ss, first_xpp=None):
            zq_prev = alloc_z(0)
            emit_transposes(0, first_xpp or load_pair_inputs(0), zq_prev)
            xpp_next2 = load_pair_inputs(1)
            for pair in range(1, BL // 2):
                zq = alloc_z(pair)
                emit_transposes(pair, xpp_next2, zq)
                if pair + 1 < BL // 2:
                    # prefetch one pair further out so the input transfer
                    # absorbs DMA-engine contention without starving the PE
                    xpp_next2 = load_pair_inputs(pair + 1)
                emit_grams(pair - 1, *zq_prev)
                if pair == BL // 2 - 1:
                    # pair 6: write now, read in the tail after ALL the G
                    # writes -- a read queued right behind its own write
                    # head-of-line blocks the sync queue on the write's
                    # completion semaphore, delaying pair 7's writes
                    emit_diag_write(pair - 1)
                else:
                    emit_diag_start(pair - 1)
                if pair >= 2:
                    emit_diag_finish(pair - 2)
                zq_prev = zq
                if inline_ss and pair <= 4:
                    emit_ss_load(pair - 1)
                if inline_ss and pair in (5, 6):
                    emit_ss_prep(2 * (pair - 5))
                    emit_ss_prep(2 * (pair - 5) + 1)
            # tail: last pair's diag round-trip is split per batch and
            # pipelined; all three outstanding G writes issue before the
            # first diagonal read, so every read dispatches with its write
            # already landed; the single-pass spectrum chases the last read,
            # with the first half of the peak compares hidden under the
            # later chunks
            last = BL // 2 - 1
            emit_grams_half(last, 0, *zq_prev)
            emit_diag_write_b(last, 0)
            emit_grams_half(last, 1, *zq_prev)
            emit_diag_write_b(last, 1)
            emit_diag_read(last - 1)
            emit_diag_read_b(last, 0)
            emit_diag_read_b(last, 1)
            emit_diag_finish(last - 1)
            emit_diag_finish(last, split=True)
            st = spectrum_begin()
            spectrum_chunks(st, [0, 1, 2, 3])
            peaks_compare(st, (0, 64), nc.vector)
            peaks_max((0, 64))
            spectrum_chunks(st, [4, 5, 6, 7])
            peaks_compare(st, (64, 128), nc.vector)
            peaks_max((64, 128))

        if loop_n is None:
            emit_body(inline_ss=True, first_xpp=xpp_next)
        else:
            for part in range(4):
                emit_ss_load(part)
            for part in range(4):
                emit_ss_prep(part)
            with tc.For_i(0, loop_n, 1):
                emit_body(inline_ss=False)

    nc.compile()
    _cache[key] = nc
    return nc


def _feeds(x_real, x_imag, steer_real, steer_imag):
    """Per-core input maps for run_bass_kernel_spmd / the timing harness."""
    ident, aux = _host_constants()
    maps = []
    for c in range(NCORES):
        sl = slice(c * BL, (c + 1) * BL)
        maps.append({
            "xr": np.ascontiguousarray(x_real[sl]),
            "xi": np.ascontiguousarray(x_imag[sl]),
            "sr": steer_real,
            "si": steer_imag,
            "ident": ident,
            "aux": aux,
        })
    return maps


def _is_ula(sr, si, atol=1e-3):
    """Check the steering matrix has the phase-additive ULA structure the
    Toeplitz reduction relies on."""
    if not (np.allclose(sr[0], 1.0, atol=atol) and np.allclose(si[0], 0.0, atol=atol)):
        return False
    # conj(S_n) * S_{n+1} should equal S_1 for every n
    re = sr[:-1] * sr[1:] + si[:-1] * si[1:]
    im = sr[:-1] * si[1:] - si[:-1] * sr[1:]
    return bool(
        np.allclose(re, sr[1][None, :], atol=atol)
        and np.allclose(im, si[1][None, :], atol=atol)
    )


def _fallback_numpy(x_real, x_imag, steer_real, steer_imag, angles, M):
    x = x_real.astype(np.float32) + 1j * x_imag.astype(np.float32)
    cov = np.matmul(x, np.conj(np.swapaxes(x, 1, 2))) / np.float32(T)
    S = steer_real.astype(np.float32) + 1j * steer_imag.astype(np.float32)
    spec = np.einsum("na,bnm,ma->ba", np.conj(S), cov, S).real.astype(np.float32)
    labels = np.zeros((spec.shape[0], M), np.float32)
    for b in range(spec.shape[0]):
        s = spec[b]
        pk = (s[1:-1] > s[:-2]) & (s[1:-1] > s[2:])
        masked = np.full(A, -np.inf, np.float32)
        masked[1:-1][pk] = s[1:-1][pk]
        order = np.argsort(-masked, kind="stable")[:M]
        labels[b] = angles[order]
    return labels


def _select_labels(spec, idx8, val8, x_real, x_imag, steer_real, steer_imag,
                   angles, M):
    """Top-M selection from device top-8 candidates with fp64 refinement of
    numerically risky (near-tie / flat-top) cases."""
    S64 = steer_real.astype(np.float64) + 1j * steer_imag.astype(np.float64)
    labels = np.zeros((B, M), np.float32)
    for b in range(B):
        cands = idx8[b].astype(np.int64)
        vals = val8[b].astype(np.float64)
        s = spec[b]
        suspect = np.zeros(8, bool)
        for j in range(7):
            if vals[j + 1] > -1e29 and (vals[j] - vals[j + 1]) < RISK_RANK * abs(vals[j]):
                suspect[j] = suspect[j + 1] = True
        flat = np.zeros(8, bool)
        for j, c in enumerate(cands):
            if 1 <= c <= A - 2 and (
                abs(s[c] - s[c - 1]) < RISK_FLAT * abs(s[c])
                or abs(s[c] - s[c + 1]) < RISK_FLAT * abs(s[c])
            ):
                suspect[j] = flat[j] = True
        if not suspect.any():
            labels[b] = angles[cands[:M]]
            continue
        # fp64 evaluation of the reference's direct quadratic form at the
        # union of suspect windows
        bins = set()
        for j in range(8):
            if flat[j]:
                for o in range(-3, 4):
                    if 0 <= cands[j] + o < A:
                        bins.add(int(cands[j] + o))
            elif suspect[j]:
                bins.add(int(cands[j]))
        bins = sorted(bins)
        x64 = x_real[b].astype(np.float64) + 1j * x_imag[b].astype(np.float64)
        Y = np.conj(x64).T @ S64[:, bins]  # (T, len(bins))
        sv = dict(zip(bins, (np.abs(Y) ** 2).sum(axis=0) / T))
        # refined candidate list: (value, device_rank, position)
        refined = []
        for j in range(8):
            c = int(cands[j])
            if vals[j] < -1e29:
                continue
            if flat[j]:
                # true local-max position near c per fp64
                best = None
                for o in range(-2, 3):
                    a = c + o
                    if a - 1 in sv and a + 1 in sv and a in sv:
                        if sv[a] > sv[a - 1] and sv[a] > sv[a + 1]:
                            if best is None or sv[a] > sv[best]:
                                best = a
                if best is None:
                    best = c
                refined.append((float(sv[best]), best))
            elif suspect[j]:
                refined.append((float(sv[c]), c))
            else:
                refined.append((float(vals[j]), c))
        # dedupe positions (two flat candidates can refine to the same bin)
        seen = {}
        for v, p in refined:
            if p not in seen or v > seen[p]:
                seen[p] = v
        order = sorted(seen.items(), key=lambda kv: (-kv[1], kv[0]))
        sel = [p for p, _ in order[:M]]
        while len(sel) < M:
            for c in cands:
                if int(c) not in sel:
                    sel.append(int(c))
                    break
        labels[b] = angles[sel]
    return labels


def kernel(x_real, x_imag, steer_real, steer_imag, angles, M):
    x_real = np.ascontiguousarray(np.asarray(x_real), dtype=np.float32)
    x_imag = np.ascontiguousarray(np.asarray(x_imag), dtype=np.float32)
    steer_real = np.ascontiguousarray(np.asarray(steer_real), dtype=np.float32)
    steer_imag = np.ascontiguousarray(np.asarray(steer_imag), dtype=np.float32)
    angles = np.asarray(angles)
    M = int(M)

    if (
        x_real.shape != (B, N, T)
        or steer_real.shape != (N, A)
        or M > 8
        or not _is_ula(steer_real, steer_imag)
    ):
        return _fallback_numpy(x_real, x_imag, steer_real, steer_imag, angles, M)

    from concourse.bass_utils import run_bass_kernel_spmd

    nc = build_program()
    in_maps = _feeds(x_real, x_imag, steer_real, steer_imag)
    res = run_bass_kernel_spmd(nc, in_maps, list(range(NCORES))).results

    spec = np.concatenate([res[c]["out_spec"] for c in range(NCORES)], axis=0)
    idx8, val8 = _merge_quarters(
        [res[c]["out_idx"] for c in range(NCORES)],
        [res[c]["out_val"] for c in range(NCORES)],
    )

    return _select_labels(
        spec, idx8, val8, x_real, x_imag, steer_real, steer_imag, angles, M
    )


def _merge_quarters(idx_list, val_list):
    """Merge per-(eighth-chunk, batch) top-8 rows into per-batch global
    top-8. Device row r of a core: eighth chunk j = r // 16, local batch
    = r % 16; local column l maps to global angle index 450*j - 1 + l.
    Non-peak filler entries are 0.0."""
    ncores = len(idx_list)
    idx8 = np.zeros((ncores * BL, 8), np.int64)
    val8 = np.full((ncores * BL, 8), -np.float64(BIG), np.float32)
    for c in range(ncores):
        iv = idx_list[c].astype(np.int64)  # (8*BL, 8)
        vv = val_list[c]
        for bl in range(BL):
            cand_v = []
            cand_i = []
            for j in range(8):
                r = 16 * j + bl
                gi = 450 * j - 1 + iv[r]
                keep = vv[r] > 0.5
                cand_v.append(vv[r][keep])
                cand_i.append(gi[keep])
            cv = np.concatenate(cand_v)
            ci = np.concatenate(cand_i)
            order = np.lexsort((ci, -cv.astype(np.float64)))[:8]
            b = c * BL + bl
            val8[b, :len(order)] = cv[order]
            idx8[b, :len(order)] = ci[order]
    return idx8, val8
